# revision 1
# baseline (speedup 1.0000x reference)
"""Bass/Trainium2 kernel for nn_JP_Featurization (gnn_message_passing).

Strategy (8 NeuronCores):
  - lg edges are range-sharded across cores by lg_src (each core owns a
    50000-wide g-edge range, so the first segment-sum is core-local).
  - NEFF-A: per-core gather of atomic_number[g_src], atomic_number[g_dst]
    for its g-edge slice (indirect DMA, 128 offsets/call).
  - NEFF-B (per core): gather pk[lg_src], k_dst[lg_dst] per lg edge,
    compute spatial symmetry (theta = pi/2 - clip(ct) exactly in fp32, so
    cos(a*theta+B) is a quadratic in ct; only Ln/Exp transcendentals),
    build 17-wide payloads (16 one-hot-weighted spatial values + count),
    segment-sum into A[g_edge,17] via dedup-matmul + CCE-add indirect DMA
    scatter (duplicates within a 128-chunk are merged by a selection-matrix
    matmul; only group leaders scatter, others go to a trash row; chunks
    rotate over 4 accumulators to avoid serialization), normalize by count,
    then scatter-mean stage 2 payloads into M[node,17] the same way.
  - Host sums the per-core M partials (data-parallel unshard).
  - NEFF-C: final (node,16) x (16,64) matmul with the reshaped value table
    and division by node counts.
"""
import math
import time

import numpy as np

import concourse.bass as bass
import concourse.bacc as bacc
import concourse.mybir as mybir
from concourse.tile import TileContext
from concourse import bass_utils

P = 128
NCORES = 8
N_NODES = 50000
N_G = 400000
N_LG = 600000
OUT_F = 64
EPS = 0.001

GPC = 50048          # padded g-edges per core (128*391)
GQ = GPC // P        # 391
GT = GPC * NCORES    # 400384 global padded pk table rows
EPC = 80000          # padded lg edges per core (128*625)
EQ = EPC // P        # 625
AQ = 393             # A/M row blocks (128*393 = 50304 rows)
AROWS = AQ * P       # 50304
TRASH = 50250        # dedup trash row (block 392, never read)
NQ = 392             # node blocks used in final phase (50176 rows)
NROWS = NQ * P
JROT = 4             # accumulator rotation depth

f32 = mybir.dt.float32
i32 = mybir.dt.int32
Alu = mybir.AluOpType
Act = mybir.ActivationFunctionType


def _nc():
    return bacc.Bacc("TRN2", target_bir_lowering=False, debug=False,
                     num_devices=NCORES)


def _gather_cols(nc, out_t, table, off_t, n):
    """n indirect gathers of 128 f32 each: out_t[:,k] = table[off_t[:,k]]."""
    for k in range(n):
        nc.gpsimd.indirect_dma_start(
            out=out_t[:, k:k + 1],
            out_offset=None,
            in_=table[:],
            in_offset=bass.IndirectOffsetOnAxis(ap=off_t[:, k:k + 1], axis=0),
        )


def build_neff_a():
    """Gather atomic[g_src], atomic[g_dst] for this core's g slice."""
    nc = _nc()
    atab = nc.dram_tensor("atab", [N_NODES, 1], f32, kind="ExternalInput")
    gs = nc.dram_tensor("gs", [P, GQ], i32, kind="ExternalInput")
    gd = nc.dram_tensor("gd", [P, GQ], i32, kind="ExternalInput")
    ks = nc.dram_tensor("ks", [P, GQ], f32, kind="ExternalOutput")
    kd = nc.dram_tensor("kd", [P, GQ], f32, kind="ExternalOutput")
    with TileContext(nc) as tc:
        with tc.tile_pool(name="sb", bufs=1) as pool:
            gs_t = pool.tile([P, GQ], i32)
            gd_t = pool.tile([P, GQ], i32)
            nc.sync.dma_start(out=gs_t[:], in_=gs[:])
            nc.sync.dma_start(out=gd_t[:], in_=gd[:])
            ks_t = pool.tile([P, GQ], f32)
            kd_t = pool.tile([P, GQ], f32)
            _gather_cols(nc, ks_t, atab, gs_t, GQ)
            _gather_cols(nc, kd_t, atab, gd_t, GQ)
            nc.sync.dma_start(out=ks[:], in_=ks_t[:])
            nc.sync.dma_start(out=kd[:], in_=kd_t[:])
    nc.compile()
    return nc


def _dedup_scatter(nc, tc, pool, psum, idxf, X_v, cols, nchunks, accs,
                   ident_t, lt_t, idxi_name):
    """Segment-sum scatter: for chunk k, merge duplicate rows via selection
    matmul, route non-leaders to TRASH, CCE-add leaders into accs[k%JROT].

    idxf: [P, nchunks] f32 destination rows. X_v: [P, nchunks, cols] payload.
    """
    r_all = pool.tile([P, nchunks], f32, tag="dedup_r")
    G_all = pool.tile([P, nchunks * cols], f32, tag="Gall")
    G_v = G_all[:].rearrange("p (q c) -> p q c", c=cols)
    for k in range(nchunks):
        idxT = psum.tile([P, P], f32, tag="idxT")
        nc.tensor.transpose(out=idxT[:],
                            in_=idxf[:, k:k + 1].to_broadcast([P, P]),
                            identity=ident_t[:])
        S = pool.tile([P, P], f32, tag="selmat")
        nc.vector.tensor_tensor(out=S[:], in0=idxf[:, k:k + 1].to_broadcast([P, P]),
                                in1=idxT[:], op=Alu.is_equal)
        L = pool.tile([P, P], f32, tag="lmat")
        nc.vector.tensor_tensor(out=L[:], in0=S[:], in1=lt_t[:], op=Alu.mult)
        nc.vector.tensor_reduce(out=r_all[:, k:k + 1], in_=L[:],
                                axis=mybir.AxisListType.X, op=Alu.add)
        Gp = psum.tile([P, cols], f32, tag="gpsum")
        nc.tensor.matmul(out=Gp[:], lhsT=S[:], rhs=X_v[:, k, :], start=True,
                         stop=True)
        nc.vector.tensor_copy(out=G_v[:, k, :], in_=Gp[:])
    # idx' = idx + min(r,1) * (TRASH - idx)
    t_m = pool.tile([P, nchunks], f32, tag="dedup_t")
    nc.vector.tensor_scalar_min(t_m[:], r_all[:], 1.0)
    diff = pool.tile([P, nchunks], f32, tag="dedup_d")
    nc.scalar.activation(out=diff[:], in_=idxf[:], func=Act.Copy,
                         bias=float(TRASH), scale=-1.0)
    nc.vector.tensor_tensor(out=t_m[:], in0=t_m[:], in1=diff[:], op=Alu.mult)
    nc.vector.tensor_tensor(out=t_m[:], in0=t_m[:], in1=idxf[:], op=Alu.add)
    idxp = pool.tile([P, nchunks], i32, tag="dedup_i")
    nc.vector.tensor_copy(out=idxp[:], in_=t_m[:])
    for k in range(nchunks):
        acc = accs[k % JROT]
        nc.gpsimd.indirect_dma_start(
            out=acc[:],
            out_offset=bass.IndirectOffsetOnAxis(ap=idxp[:, k:k + 1], axis=0),
            in_=G_v[:, k, :],
            in_offset=None,
            compute_op=Alu.add,
        )


def build_neff_b(sc):
    """Main per-core kernel. sc: dict of spatial scalar constants."""
    nc = _nc()
    pk_tab = nc.dram_tensor("pk_tab", [GT, 1], f32, kind="ExternalInput")
    lgs_g = nc.dram_tensor("lgs_g", [P, EQ], i32, kind="ExternalInput")
    lgd_g = nc.dram_tensor("lgd_g", [P, EQ], i32, kind="ExternalInput")
    lgs_l = nc.dram_tensor("lgs_l", [P, EQ], i32, kind="ExternalInput")
    ct_in = nc.dram_tensor("ct", [P, EQ], f32, kind="ExternalInput")
    dnr_in = nc.dram_tensor("dnr", [P, EQ], f32, kind="ExternalInput")
    gsrc = nc.dram_tensor("gsrc", [P, GQ], i32, kind="ExternalInput")
    gmask = nc.dram_tensor("gmask", [P, GQ], f32, kind="ExternalInput")
    ident = nc.dram_tensor("ident", [P, P], f32, kind="ExternalInput")
    ltri = nc.dram_tensor("ltri", [P, P], f32, kind="ExternalInput")
    m_out = nc.dram_tensor("m_out", [AROWS, 17], f32, kind="ExternalOutput")

    with TileContext(nc) as tc:
        with (
            tc.tile_pool(name="sb", bufs=1) as pool,
            tc.tile_pool(name="ps", bufs=4, space="PSUM") as psum,
            tc.tile_pool(name="dr", bufs=1, space="DRAM") as dram,
        ):
            # accumulators in DRAM, zero-initialized
            A_js = [dram.tile([AROWS, 17], f32, tag=f"A{j}", name=f"Aacc{j}") for j in range(JROT)]
            M_js = [dram.tile([AROWS, 17], f32, tag=f"M{j}", name=f"Macc{j}") for j in range(JROT)]
            zt = pool.tile([P, AQ * 17], f32, tag="accsum")
            nc.vector.memset(zt[:], 0.0)
            for j in range(JROT):
                nc.sync.dma_start(
                    out=A_js[j][:].rearrange("(q p) c -> p q c", p=P),
                    in_=zt[:].rearrange("p (q c) -> p q c", c=17))
                nc.sync.dma_start(
                    out=M_js[j][:].rearrange("(q p) c -> p q c", p=P),
                    in_=zt[:].rearrange("p (q c) -> p q c", c=17))

            ident_t = pool.tile([P, P], f32)
            lt_t = pool.tile([P, P], f32)
            nc.sync.dma_start(out=ident_t[:], in_=ident[:])
            nc.sync.dma_start(out=lt_t[:], in_=ltri[:])

            lgs_g_t = pool.tile([P, EQ], i32, tag="lgs_g_t")
            lgd_g_t = pool.tile([P, EQ], i32, tag="lgd_g_t")
            lgs_l_t = pool.tile([P, EQ], i32)
            ct_t = pool.tile([P, EQ], f32)
            dnr_t = pool.tile([P, EQ], f32)
            for t, src in ((lgs_g_t, lgs_g), (lgd_g_t, lgd_g), (lgs_l_t, lgs_l),
                           (ct_t, ct_in), (dnr_t, dnr_in)):
                nc.sync.dma_start(out=t[:], in_=src[:])

            # ---- P1: per-edge gathers ----
            pk1 = pool.tile([P, EQ], f32)
            pk2 = pool.tile([P, EQ], f32)
            _gather_cols(nc, pk1, pk_tab, lgs_g_t, EQ)
            _gather_cols(nc, pk2, pk_tab, lgd_g_t, EQ)
            # kc = floor(pk2/4) via threshold masks
            kc = pool.tile([P, EQ], f32)
            t4 = pool.tile([P, EQ], f32, tag="unpk2")
            nc.vector.tensor_scalar(out=kc[:], in0=pk2[:], scalar1=4.0,
                                    scalar2=None, op0=Alu.is_ge)
            nc.vector.tensor_scalar(out=t4[:], in0=pk2[:], scalar1=8.0,
                                    scalar2=None, op0=Alu.is_ge)
            nc.vector.tensor_tensor(out=kc[:], in0=kc[:], in1=t4[:], op=Alu.add)
            nc.vector.tensor_scalar(out=t4[:], in0=pk2[:], scalar1=12.0,
                                    scalar2=None, op0=Alu.is_ge)
            nc.vector.tensor_tensor(out=kc[:], in0=kc[:], in1=t4[:], op=Alu.add)

            # unpack pk1 = ka + 4*kb
            # kb = floor(pk1/4) via threshold masks (pk1 in 0..15)
            ka = pool.tile([P, EQ], f32)
            kb = pool.tile([P, EQ], f32)
            tmp = pool.tile([P, EQ], f32, tag="unpk")
            nc.vector.tensor_scalar(out=kb[:], in0=pk1[:], scalar1=4.0,
                                    scalar2=None, op0=Alu.is_ge)
            nc.vector.tensor_scalar(out=tmp[:], in0=pk1[:], scalar1=8.0,
                                    scalar2=None, op0=Alu.is_ge)
            nc.vector.tensor_tensor(out=kb[:], in0=kb[:], in1=tmp[:], op=Alu.add)
            nc.vector.tensor_scalar(out=tmp[:], in0=pk1[:], scalar1=12.0,
                                    scalar2=None, op0=Alu.is_ge)
            nc.vector.tensor_tensor(out=kb[:], in0=kb[:], in1=tmp[:], op=Alu.add)
            # ka = pk1 - 4*kb
            nc.vector.tensor_scalar_mul(tmp[:], kb[:], -4.0)
            nc.vector.tensor_tensor(out=ka[:], in0=pk1[:], in1=tmp[:], op=Alu.add)

            periph = pool.tile([P, EQ], f32)
            nc.vector.tensor_tensor(out=periph[:], in0=ka[:], in1=kc[:],
                                    op=Alu.is_equal)
            c1 = pool.tile([P, EQ], f32)
            nc.vector.tensor_tensor(out=c1[:], in0=kb[:], in1=ka[:],
                                    op=Alu.is_equal)
            c2 = ka
            nc.vector.tensor_tensor(out=c2[:], in0=kb[:], in1=kc[:],
                                    op=Alu.is_equal)
            nc.vector.tensor_tensor(out=c1[:], in0=c1[:], in1=c2[:], op=Alu.mult)
            sym = kc
            nc.vector.tensor_scalar_mul(sym[:], periph[:], 2.0)
            nc.vector.tensor_tensor(out=sym[:], in0=sym[:], in1=c1[:], op=Alu.add)

            # ---- spatial ----
            x = ct_t
            nc.vector.tensor_scalar_min(x[:], ct_t[:], EPS)
            nc.vector.tensor_scalar_max(x[:], x[:], -EPS)
            x2 = pool.tile([P, EQ], f32, tag="x2sh")
            nc.vector.tensor_tensor(out=x2[:], in0=x[:], in1=x[:], op=Alu.mult)
            dnr2 = dnr_t
            nc.vector.tensor_tensor(out=dnr2[:], in0=dnr_t[:], in1=dnr_t[:],
                                    op=Alu.mult)
            sps = []
            for h in range(4):
                y = pool.tile([P, EQ], f32, tag=f"y{h}")
                nc.scalar.activation(out=y[:], in_=x[:], func=Act.Copy,
                                     bias=sc["q0"][h], scale=sc["q1"][h])
                t2 = pool.tile([P, EQ], f32, tag="sptmp")
                nc.vector.tensor_scalar_mul(t2[:], x2[:], sc["q2"][h])
                nc.vector.tensor_tensor(out=y[:], in0=y[:], in1=t2[:], op=Alu.add)
                nc.scalar.activation(out=y[:], in_=y[:], func=Act.Ln, bias=0.0,
                                     scale=1.0)
                # z = c_h * ln(y) - d_h * dnr2
                nc.vector.tensor_scalar_mul(y[:], y[:], sc["c"][h])
                nc.vector.tensor_scalar_mul(t2[:], dnr2[:], sc["d"][h])
                nc.vector.tensor_tensor(out=y[:], in0=y[:], in1=t2[:],
                                        op=Alu.subtract)
                nc.scalar.activation(out=y[:], in_=y[:], func=Act.Exp, bias=0.0,
                                     scale=1.0)
                sps.append(y)

            # ---- payload X [P, EQ, 17] ----
            X = pool.tile([P, EQ * 17], f32, tag="payload")
            X_v = X[:].rearrange("p (q c) -> p q c", c=17)
            for kk in range(4):
                m = pool.tile([P, EQ], f32, tag="x2sh")
                nc.vector.tensor_scalar(out=m[:], in0=sym[:], scalar1=float(kk),
                                        scalar2=None, op0=Alu.is_equal)
                for h in range(4):
                    nc.vector.tensor_tensor(out=X_v[:, :, kk * 4 + h], in0=m[:],
                                            in1=sps[h][:], op=Alu.mult)
            nc.vector.memset(X_v[:, :, 16], 1.0)

            # ---- S1 scatter: A[lgs_l] += X ----
            idxf1 = pool.tile([P, EQ], f32, tag="lgs_g_t")
            nc.vector.tensor_copy(out=idxf1[:], in_=lgs_l_t[:])
            _dedup_scatter(nc, tc, pool, psum, idxf1, X_v, 17, EQ, A_js,
                           ident_t, lt_t, "s1")

            # ---- Abar = A[:, :16] / max(cnt,1) ----
            Asum = pool.tile([P, AQ * 17], f32, tag="accsum")
            nc.sync.dma_start(out=Asum[:].rearrange("p (q c) -> p q c", c=17),
                              in_=A_js[0][:].rearrange("(q p) c -> p q c", p=P))
            for j in range(1, JROT):
                tj = pool.tile([P, AQ * 17], f32, tag="accld")
                nc.sync.dma_start(
                    out=tj[:].rearrange("p (q c) -> p q c", c=17),
                    in_=A_js[j][:].rearrange("(q p) c -> p q c", p=P))
                nc.vector.tensor_tensor(out=Asum[:], in0=Asum[:], in1=tj[:],
                                        op=Alu.add)
            As_v = Asum[:].rearrange("p (q c) -> p q c", c=17)
            cnt = pool.tile([P, AQ], f32)
            nc.vector.tensor_copy(out=cnt[:], in_=As_v[:, :, 16])
            nc.vector.tensor_scalar_max(cnt[:], cnt[:], 1.0)
            inv = pool.tile([P, AQ], f32)
            nc.vector.reciprocal(out=inv[:], in_=cnt[:])
            # one Newton step: inv = inv*(2 - cnt*inv)
            nt = pool.tile([P, AQ], f32)
            nc.vector.tensor_tensor(out=nt[:], in0=cnt[:], in1=inv[:], op=Alu.mult)
            nc.scalar.activation(out=nt[:], in_=nt[:], func=Act.Copy, bias=2.0,
                                 scale=-1.0)
            nc.vector.tensor_tensor(out=inv[:], in0=inv[:], in1=nt[:], op=Alu.mult)

            # ---- stage-2 payload Y [P, GQ, 17] ----
            Y = pool.tile([P, GQ * 17], f32, tag="payload")
            Y_v = Y[:].rearrange("p (q c) -> p q c", c=17)
            for c in range(16):
                nc.vector.tensor_tensor(out=Y_v[:, :, c], in0=As_v[:, :GQ, c],
                                        in1=inv[:, :GQ], op=Alu.mult)
            gm_t = pool.tile([P, GQ], f32)
            nc.sync.dma_start(out=gm_t[:], in_=gmask[:])
            nc.vector.tensor_copy(out=Y_v[:, :, 16], in_=gm_t[:])

            # ---- S2 scatter: M[gsrc] += Y ----
            gsrc_t = pool.tile([P, GQ], i32)
            nc.sync.dma_start(out=gsrc_t[:], in_=gsrc[:])
            idxf2 = pool.tile([P, GQ], f32, tag="lgd_g_t")
            nc.vector.tensor_copy(out=idxf2[:], in_=gsrc_t[:])
            _dedup_scatter(nc, tc, pool, psum, idxf2, Y_v, 17, GQ, M_js,
                           ident_t, lt_t, "s2")

            # ---- M sum -> out ----
            Msum = pool.tile([P, AQ * 17], f32, tag="accsum")
            nc.sync.dma_start(out=Msum[:].rearrange("p (q c) -> p q c", c=17),
                              in_=M_js[0][:].rearrange("(q p) c -> p q c", p=P))
            for j in range(1, JROT):
                tj = pool.tile([P, AQ * 17], f32, tag="accld")
                nc.sync.dma_start(
                    out=tj[:].rearrange("p (q c) -> p q c", c=17),
                    in_=M_js[j][:].rearrange("(q p) c -> p q c", p=P))
                nc.vector.tensor_tensor(out=Msum[:], in0=Msum[:], in1=tj[:],
                                        op=Alu.add)
            nc.sync.dma_start(out=m_out[:].rearrange("(q p) c -> p q c", p=P),
                              in_=Msum[:].rearrange("p (q c) -> p q c", c=17))
    nc.compile()
    return nc


def build_neff_c():
    """out[n,:] = (M[n,:16] @ VT2) / max(M[n,16],1)."""
    nc = _nc()
    m_in = nc.dram_tensor("m_in", [AROWS, 17], f32, kind="ExternalInput")
    vt4 = nc.dram_tensor("vt4", [64, 256], f32, kind="ExternalInput")
    ident = nc.dram_tensor("ident", [P, P], f32, kind="ExternalInput")
    out = nc.dram_tensor("out", [NROWS, OUT_F], f32, kind="ExternalOutput")
    with TileContext(nc) as tc:
        with (
            tc.tile_pool(name="sb", bufs=2) as pool,
            tc.tile_pool(name="ps", bufs=4, space="PSUM") as psum,
        ):
            ident_t = pool.tile([P, P], f32)
            nc.sync.dma_start(out=ident_t[:], in_=ident[:])
            vt_t = pool.tile([64, 256], f32)
            nc.sync.dma_start(out=vt_t[:], in_=vt4[:])
            M_t = pool.tile([P, AQ * 17], f32)
            nc.sync.dma_start(out=M_t[:].rearrange("p (q c) -> p q c", c=17),
                              in_=m_in[:].rearrange("(q p) c -> p q c", p=P))
            M_v = M_t[:].rearrange("p (q c) -> p q c", c=17)
            cnt = pool.tile([P, NQ], f32)
            nc.vector.tensor_copy(out=cnt[:], in_=M_v[:, :NQ, 16])
            nc.vector.tensor_scalar_max(cnt[:], cnt[:], 1.0)
            inv = pool.tile([P, NQ], f32)
            nc.vector.reciprocal(out=inv[:], in_=cnt[:])
            nt = pool.tile([P, NQ], f32)
            nc.vector.tensor_tensor(out=nt[:], in0=cnt[:], in1=inv[:], op=Alu.mult)
            nc.scalar.activation(out=nt[:], in_=nt[:], func=Act.Copy, bias=2.0,
                                 scale=-1.0)
            nc.vector.tensor_tensor(out=inv[:], in0=inv[:], in1=nt[:], op=Alu.mult)

            # gather the 16 value cols of 4 node-blocks into [P, 64]
            out_v = out[:].rearrange("(q p) f -> p q f", p=P)
            for b in range(NQ // 4):
                blk = pool.tile([P, 64], f32, tag="blk")
                for t in range(4):
                    nc.vector.tensor_copy(out=blk[:, t * 16:(t + 1) * 16],
                                          in_=M_v[:, 4 * b + t, 0:16])
                tp = psum.tile([64, P], f32, tag="tp")
                nc.tensor.transpose(out=tp[:], in_=blk[:], identity=ident_t[:])
                tps = pool.tile([64, P], f32, tag="tps")
                nc.vector.tensor_copy(out=tps[:], in_=tp[:])
                op = psum.tile([P, 256], f32, tag="op")
                nc.tensor.matmul(out=op[:], lhsT=tps[:], rhs=vt_t[:], start=True,
                                 stop=True)
                ob = pool.tile([P, 256], f32, tag="ob")
                for t in range(4):
                    nc.vector.tensor_tensor(
                        out=ob[:, t * 64:(t + 1) * 64],
                        in0=op[:, t * 64:(t + 1) * 64],
                        in1=inv[:, 4 * b + t:4 * b + t + 1].to_broadcast([P, 64]),
                        op=Alu.mult)
                nc.sync.dma_start(out=out_v[:, 4 * b:4 * b + 4, :],
                                  in_=ob[:].rearrange("p (q f) -> p q f", f=64))
    nc.compile()
    return nc


_CACHE = {}


def kernel(atomic_number, g_src, g_dst, lg_src, lg_dst, costheta, dnr, a, b, c,
           d, value_table):
    atomic_number = np.asarray(atomic_number)
    g_src = np.asarray(g_src).astype(np.int64)
    g_dst = np.asarray(g_dst).astype(np.int64)
    lg_src = np.asarray(lg_src).astype(np.int64)
    lg_dst = np.asarray(lg_dst).astype(np.int64)
    costheta = np.asarray(costheta, dtype=np.float32)
    dnr = np.asarray(dnr, dtype=np.float32)
    a = np.asarray(a, dtype=np.float64)
    b = np.asarray(b, dtype=np.float64)
    c = np.asarray(c, dtype=np.float64)
    d = np.asarray(d, dtype=np.float64)
    value_table = np.asarray(value_table, dtype=np.float32)

    cores = list(range(NCORES))
    hw_ns = [0.0]

    def run(nc, in_maps, core_ids):
        t0 = time.time()
        res = bass_utils.run_bass_kernel_spmd(nc, in_maps, core_ids=core_ids)
        wall_ns = (time.time() - t0) * 1e9
        # exec_time_ns requires an NTFF trace (unavailable under axon here);
        # fall back to dispatch wall time as an upper bound.
        hw_ns[0] += res.exec_time_ns if res.exec_time_ns else wall_ns
        return res.results

    # ---------------- NEFF A: build pk tables ----------------
    if "A" not in _CACHE:
        _CACHE["A"] = build_neff_a()
    atab = atomic_number.astype(np.float32).reshape(N_NODES, 1)
    in_maps = []
    for ci in cores:
        gs = np.zeros(GPC, np.int32)
        gd = np.zeros(GPC, np.int32)
        sl = slice(ci * N_NODES, (ci + 1) * N_NODES)
        gs[:N_NODES] = g_src[sl]
        gd[:N_NODES] = g_dst[sl]
        in_maps.append({"atab": atab, "gs": gs.reshape(P, GQ),
                        "gd": gd.reshape(P, GQ)})
    resA = run(_CACHE["A"], in_maps, cores)
    ks_full = np.concatenate([r["ks"].reshape(-1) for r in resA])  # [GT]
    kd_full = np.concatenate([r["kd"].reshape(-1) for r in resA])
    pk_tab = (ks_full + 4.0 * kd_full).astype(np.float32).reshape(GT, 1)

    # ---------------- spatial scalar constants ----------------
    Ch = a * (math.pi / 2.0) + np.mod(b, math.pi)
    cosC, sinC = np.cos(Ch), np.sin(Ch)
    k0 = (cosC + 1.0) / 2.0
    k1 = sinC / 2.0
    k2 = -cosC / 4.0
    sc = {
        "q0": [float(v) for v in k0],
        "q1": [float(v) for v in k1 * a],
        "q2": [float(v) for v in k2 * a * a],
        "c": [float(v) for v in c],
        "d": [float(v) for v in d],
    }
    key = ("B",) + tuple(sc["q0"] + sc["q1"] + sc["q2"] + sc["c"] + sc["d"])
    if key not in _CACHE:
        _CACHE[key] = build_neff_b(sc)

    ident = np.eye(P, dtype=np.float32)
    ltri = np.tril(np.ones((P, P), np.float32), -1)

    # ---------------- shard lg edges by lg_src range ----------------
    owner = lg_src // N_NODES
    in_maps = []
    for ci in cores:
        sel = np.where(owner == ci)[0]
        n = len(sel)
        assert n <= EPC, f"core {ci} got {n} edges"
        ls = lg_src[sel]
        ld = lg_dst[sel]
        lgs_l = np.full(EPC, TRASH, np.int32)
        lgs_l[:n] = ls - ci * N_NODES
        lgs_gv = np.zeros(EPC, np.int32)
        lgs_gv[:n] = (ls // N_NODES) * GPC + ls % N_NODES
        lgd_gv = np.zeros(EPC, np.int32)
        lgd_gv[:n] = (ld // N_NODES) * GPC + ld % N_NODES
        ct_s = np.zeros(EPC, np.float32)
        ct_s[:n] = costheta[sel]
        dnr_s = np.zeros(EPC, np.float32)
        dnr_s[:n] = dnr[sel]
        gsrc_s = np.zeros(GPC, np.int32)
        gsrc_s[:N_NODES] = g_src[ci * N_NODES:(ci + 1) * N_NODES]
        gmask_s = np.zeros(GPC, np.float32)
        gmask_s[:N_NODES] = 1.0
        in_maps.append({
            "pk_tab": pk_tab,
            "lgs_g": lgs_gv.reshape(P, EQ), "lgd_g": lgd_gv.reshape(P, EQ),
            "lgs_l": lgs_l.reshape(P, EQ),
            "ct": ct_s.reshape(P, EQ), "dnr": dnr_s.reshape(P, EQ),
            "gsrc": np.ascontiguousarray(gsrc_s.reshape(GQ, P).T),
            "gmask": np.ascontiguousarray(gmask_s.reshape(GQ, P).T),
            "ident": ident, "ltri": ltri,
        })
    resB = run(_CACHE[key], in_maps, cores)
    M_red = np.zeros((AROWS, 17), np.float32)
    for r in resB:
        M_red += r["m_out"]

    # ---------------- NEFF C: final matmul ----------------
    if "C" not in _CACHE:
        _CACHE["C"] = build_neff_c()
    # vt4 = blockdiag of VT2 (16x64) x4; VT2[k*4+h, f] = value_table[k, f*4+h]
    VT2 = value_table.reshape(4, OUT_F, 4).transpose(0, 2, 1).reshape(16, OUT_F)
    vt4 = np.zeros((64, 256), np.float32)
    for t in range(4):
        vt4[t * 16:(t + 1) * 16, t * 64:(t + 1) * 64] = VT2
    resC = run(_CACHE["C"], [{"m_in": M_red, "vt4": vt4, "ident": ident}], [0])
    out = resC[0]["out"][:N_NODES]
    kernel.last_hw_ns = hw_ns[0]
    return out.astype(np.float32)



# revision 3
# speedup vs baseline: 2.0247x; 2.0247x over previous
"""Bass/Trainium2 kernel for nn_JP_Featurization (gnn_message_passing).

Single fused SPMD NEFF on 8 cores (one run_bass_kernel_spmd dispatch):
  - g-edges are range-sharded (50000/core). Each core gathers
    atomic[g_src], atomic[g_dst] for its slice, packs pk = ks + 4*kd,
    and an in-kernel AllGather forms the full per-g-edge pk table.
  - lg edges are sharded by owner of lg_src. The host sorts each core's
    edges by destination g-edge *rank* (g-edges sorted by g_src), lays
    ranks out in 256-slot node blocks (row2 = 256*blk + slot), and cuts
    the edge stream into 128-edge chunks that stay inside one 128-row
    half-block. On device, a For_i loop gathers pk[lg_src], pk[lg_dst]
    per chunk; wide vector ops compute the symmetry class and spatial
    weights; a second For_i loop bins each chunk onto its 128 dest rows
    with a one-hot matmul and scatters contiguously (base+iota offsets,
    duplicate-free within a DMA) into a DRAM accumulator with CCE-add.
  - Stage 2 needs no scatter: ranks are sorted by g_src, so each node
    block's rows live in exactly 2 chunks; a For_i loop bins them with
    one-hot matmuls accumulated in PSUM (start/stop) into M[node,17].
  - ReduceScatter sums M across cores and hands each core 6272 node
    rows; a final For_i loop does (M[:, :16]/deg) @ VT2 per 128-row
    block and writes the core's output shard.
  - Host work is index-only: sharding, sorting, chunk/slot layout.
All loops are hardware For_i loops, keeping the BIR at ~200 instructions
(dispatch cost here scales with BIR size: it is re-serialized on every
jit trace).
"""
import math

import numpy as np

import concourse.bass as bass
import concourse.bacc as bacc
import concourse.mybir as mybir
from concourse.tile import TileContext
from concourse.masks import make_identity
from concourse import bass_utils

P = 128
NCORES = 8
N_NODES = 50000
N_G = 400000
N_LG = 600000
OUT_F = 64
EPS = 0.001

GSLICE = N_G // NCORES      # 50000 g-edges per core
GQ = 392                    # phase-0 gather columns (392*128 = 50176)
GPC = GQ * P                # padded pk rows per core
GT = GPC * NCORES           # full pk table rows
NBLK = 392                  # node blocks of 128 (50176 >= 50000)
SLOT2 = 256                 # stage-2 rank slots per node block
NCH2 = NBLK * 2             # stage-2 chunks (784)
AROWS = NBLK * SLOT2 + P    # A accumulator rows (+128 scatter overrun pad)
ORPC = 6272                 # output rows per core (49*128); 8*6272 = 50176
OQ = ORPC // P              # 49

f32 = mybir.dt.float32
bf16 = mybir.dt.bfloat16
i32 = mybir.dt.int32
Alu = mybir.AluOpType
Act = mybir.ActivationFunctionType


def build_fused(sc, nch1):
    nc = bacc.Bacc("TRN2", target_bir_lowering=False, debug=False,
                   num_devices=NCORES)
    atab = nc.dram_tensor("atab", [GPC, 1], f32, kind="ExternalInput")
    gs = nc.dram_tensor("gs", [P, GQ], i32, kind="ExternalInput")
    gd = nc.dram_tensor("gd", [P, GQ], i32, kind="ExternalInput")
    lgs = nc.dram_tensor("lgs", [P, nch1], i32, kind="ExternalInput")
    lgd = nc.dram_tensor("lgd", [P, nch1], i32, kind="ExternalInput")
    ct_in = nc.dram_tensor("ct", [P, nch1], f32, kind="ExternalInput")
    dnr_in = nc.dram_tensor("dnr", [P, nch1], f32, kind="ExternalInput")
    rloc = nc.dram_tensor("rloc", [P, nch1], f32, kind="ExternalInput")
    obase = nc.dram_tensor("obase", [P, nch1], i32, kind="ExternalInput")
    rloc2 = nc.dram_tensor("rloc2", [P, NCH2], f32, kind="ExternalInput")
    vmask = nc.dram_tensor("vmask", [P, NCH2], f32, kind="ExternalInput")
    vt2 = nc.dram_tensor("vt2", [16, OUT_F], f32, kind="ExternalInput")
    out_o = nc.dram_tensor("out", [ORPC, OUT_F], f32, kind="ExternalOutput")

    rg = [list(range(NCORES))]

    with TileContext(nc) as tc:
        with (
            tc.tile_pool(name="sb", bufs=1) as pool,
            tc.tile_pool(name="ps", bufs=2, space="PSUM") as psum,
            tc.tile_pool(name="dr", bufs=1, space="DRAM") as dram,
        ):
            # ---- constants generated on device ----
            iota_i = pool.tile([P, P], i32)
            nc.gpsimd.iota(iota_i[:], pattern=[[1, P]], base=0,
                           channel_multiplier=0)
            iota_b = pool.tile([P, P], bf16)
            nc.vector.tensor_copy(out=iota_b[:], in_=iota_i[:])
            iota_f = pool.tile([P, P], f32)
            nc.vector.tensor_copy(out=iota_f[:], in_=iota_i[:])
            ident_t = pool.tile([P, P], f32)
            make_identity(nc, ident_t[:])

            # ---- inputs to SBUF ----
            gs_t = pool.tile([P, GQ], i32)
            gd_t = pool.tile([P, GQ], i32)
            lgs_t = pool.tile([P, nch1], i32)
            lgd_t = pool.tile([P, nch1], i32)
            ct_t = pool.tile([P, nch1], f32)
            dnr_t = pool.tile([P, nch1], f32)
            rl_t = pool.tile([P, nch1], f32)
            ob_t = pool.tile([P, nch1], i32)
            rl2_t = pool.tile([P, NCH2], f32)
            vm_t = pool.tile([P, NCH2], f32)
            vt_t = pool.tile([16, OUT_F], f32)
            for t, s in ((gs_t, gs), (gd_t, gd), (lgs_t, lgs), (lgd_t, lgd),
                         (ct_t, ct_in), (dnr_t, dnr_in), (rl_t, rloc),
                         (ob_t, obase), (rl2_t, rloc2), (vm_t, vmask),
                         (vt_t, vt2)):
                nc.sync.dma_start(out=t[:], in_=s[:])

            # ---- DRAM bounce tensors ----
            pk_slice = dram.tile([GPC, 1], f32, name="pk_slice")
            pk_full = dram.tile([GT, 1], f32, name="pk_full",
                                addr_space="Shared")
            A_dram = dram.tile([AROWS, 17], f32, name="A_dram")
            M_dram = dram.tile([NBLK * P, 17], f32, name="M_dram")
            Mred = dram.tile([ORPC, 17], f32, name="Mred")

            # ---- phase 0: per-slice atomic gathers -> pk table ----
            ks_t = pool.tile([P, GQ], f32)
            kd_t = pool.tile([P, GQ], f32)
            o1 = pool.tile([P, 1], i32, tag="st_o1")
            o2 = pool.tile([P, 1], i32, tag="st_o2")
            g1 = pool.tile([P, 1], f32, tag="st_g1")
            g2 = pool.tile([P, 1], f32, tag="st_g2")
            with tc.For_i(0, GQ) as q:
                nc.vector.tensor_copy(out=o1[:], in_=gs_t[:, bass.ds(q, 1)])
                nc.gpsimd.indirect_dma_start(
                    out=g1[:], out_offset=None, in_=atab[:],
                    in_offset=bass.IndirectOffsetOnAxis(ap=o1[:], axis=0))
                nc.vector.tensor_copy(out=ks_t[:, bass.ds(q, 1)], in_=g1[:])
                nc.vector.tensor_copy(out=o2[:], in_=gd_t[:, bass.ds(q, 1)])
                nc.gpsimd.indirect_dma_start(
                    out=g2[:], out_offset=None, in_=atab[:],
                    in_offset=bass.IndirectOffsetOnAxis(ap=o2[:], axis=0))
                nc.vector.tensor_copy(out=kd_t[:, bass.ds(q, 1)], in_=g2[:])
            nc.vector.tensor_scalar_mul(kd_t[:], kd_t[:], 4.0)
            nc.vector.tensor_tensor(out=ks_t[:], in0=ks_t[:], in1=kd_t[:],
                                    op=Alu.add)
            nc.sync.dma_start(
                out=pk_slice[:].rearrange("(q p) c -> p q c", p=P),
                in_=ks_t[:].rearrange("p (q c) -> p q c", c=1))
            nc.gpsimd.collective_compute(
                "AllGather", Alu.bypass, replica_groups=rg,
                ins=[pk_slice[:]], outs=[pk_full[:]])

            # ---- L1: per-edge pk gathers ----
            pk1_t = pool.tile([P, nch1], f32)
            pk2_t = pool.tile([P, nch1], f32)
            with tc.For_i(0, nch1) as k:
                nc.vector.tensor_copy(out=o1[:], in_=lgs_t[:, bass.ds(k, 1)])
                nc.gpsimd.indirect_dma_start(
                    out=g1[:], out_offset=None, in_=pk_full[:],
                    in_offset=bass.IndirectOffsetOnAxis(ap=o1[:], axis=0))
                nc.vector.tensor_copy(out=pk1_t[:, bass.ds(k, 1)], in_=g1[:])
                nc.vector.tensor_copy(out=o2[:], in_=lgd_t[:, bass.ds(k, 1)])
                nc.gpsimd.indirect_dma_start(
                    out=g2[:], out_offset=None, in_=pk_full[:],
                    in_offset=bass.IndirectOffsetOnAxis(ap=o2[:], axis=0))
                nc.vector.tensor_copy(out=pk2_t[:, bass.ds(k, 1)], in_=g2[:])

            # ---- symmetry class (wide) ----
            # kb = floor(pk1/4), ka = pk1 - 4*kb, kc = floor(pk2/4)
            kb_t = pool.tile([P, nch1], f32, tag="wA")
            kc_t = pool.tile([P, nch1], f32, tag="wB")
            tmp_t = pool.tile([P, nch1], f32, tag="wC")
            sym_t = pool.tile([P, nch1], f32, tag="wD")
            for dst, src in ((kb_t, pk1_t), (kc_t, pk2_t)):
                nc.vector.tensor_scalar(out=dst[:], in0=src[:], scalar1=4.0,
                                        scalar2=None, op0=Alu.is_ge)
                nc.vector.tensor_scalar(out=tmp_t[:], in0=src[:], scalar1=8.0,
                                        scalar2=None, op0=Alu.is_ge)
                nc.vector.tensor_tensor(out=dst[:], in0=dst[:], in1=tmp_t[:],
                                        op=Alu.add)
                nc.vector.tensor_scalar(out=tmp_t[:], in0=src[:], scalar1=12.0,
                                        scalar2=None, op0=Alu.is_ge)
                nc.vector.tensor_tensor(out=dst[:], in0=dst[:], in1=tmp_t[:],
                                        op=Alu.add)
            nc.vector.tensor_scalar_mul(tmp_t[:], kb_t[:], -4.0)
            nc.vector.tensor_tensor(out=pk1_t[:], in0=pk1_t[:], in1=tmp_t[:],
                                    op=Alu.add)  # pk1_t now holds ka
            nc.vector.tensor_tensor(out=sym_t[:], in0=pk1_t[:], in1=kc_t[:],
                                    op=Alu.is_equal)  # peripheral
            nc.vector.tensor_tensor(out=pk1_t[:], in0=kb_t[:], in1=pk1_t[:],
                                    op=Alu.is_equal)  # c1 = (kb==ka)
            nc.vector.tensor_tensor(out=kb_t[:], in0=kb_t[:], in1=kc_t[:],
                                    op=Alu.is_equal)  # c2 = (kb==kc)
            nc.vector.tensor_tensor(out=pk1_t[:], in0=pk1_t[:], in1=kb_t[:],
                                    op=Alu.mult)      # central
            nc.vector.tensor_scalar_mul(sym_t[:], sym_t[:], 2.0)
            nc.vector.tensor_tensor(out=sym_t[:], in0=sym_t[:], in1=pk1_t[:],
                                    op=Alu.add)       # sym in {0,1,2,3}

            # ---- spatial weights + payload X (bf16) ----
            # theta = pi/2 - asin(clip(ct)); cos(a*theta + B) ~ quadratic in ct
            x = ct_t
            nc.vector.tensor_scalar_min(x[:], ct_t[:], EPS)
            nc.vector.tensor_scalar_max(x[:], x[:], -EPS)
            x2_t = pool.tile([P, nch1], f32, tag="wA")
            nc.vector.tensor_tensor(out=x2_t[:], in0=x[:], in1=x[:],
                                    op=Alu.mult)
            dnr2 = dnr_t
            nc.vector.tensor_tensor(out=dnr2[:], in0=dnr_t[:], in1=dnr_t[:],
                                    op=Alu.mult)
            X = pool.tile([P, nch1 * 17], bf16, tag="X")
            X_v = X[:].rearrange("p (q c) -> p q c", c=17)
            y_t = pool.tile([P, nch1], f32, tag="wB")
            t2_t = pool.tile([P, nch1], f32, tag="wC")
            m_t = pool.tile([P, nch1], f32, tag="wE")
            for h in range(4):
                nc.scalar.activation(out=y_t[:], in_=x[:], func=Act.Copy,
                                     bias=sc["q0"][h], scale=sc["q1"][h])
                nc.vector.tensor_scalar_mul(t2_t[:], x2_t[:], sc["q2"][h])
                nc.vector.tensor_tensor(out=y_t[:], in0=y_t[:], in1=t2_t[:],
                                        op=Alu.add)
                nc.scalar.activation(out=y_t[:], in_=y_t[:], func=Act.Ln,
                                     bias=0.0, scale=1.0)
                nc.vector.tensor_scalar_mul(y_t[:], y_t[:], sc["c"][h])
                nc.vector.tensor_scalar_mul(t2_t[:], dnr2[:], sc["d"][h])
                nc.vector.tensor_tensor(out=y_t[:], in0=y_t[:], in1=t2_t[:],
                                        op=Alu.subtract)
                nc.scalar.activation(out=y_t[:], in_=y_t[:], func=Act.Exp,
                                     bias=0.0, scale=1.0)
                for kk in range(4):
                    nc.vector.tensor_scalar(out=m_t[:], in0=sym_t[:],
                                            scalar1=float(kk), scalar2=None,
                                            op0=Alu.is_equal)
                    nc.vector.tensor_tensor(out=X_v[:, :, kk * 4 + h],
                                            in0=m_t[:], in1=y_t[:],
                                            op=Alu.mult)
            nc.vector.memset(X_v[:, :, 16], 1.0)

            # ---- zero A accumulator ----
            AQ = AROWS // P  # 786
            zt = pool.tile([P, AQ * 17], f32, tag="bigA")
            nc.vector.memset(zt[:], 0.0)
            nc.sync.dma_start(
                out=A_dram[:].rearrange("(q p) c -> p q c", p=P),
                in_=zt[:].rearrange("p (q c) -> p q c", c=17))

            # ---- L2: one-hot binning matmul + contiguous CCE scatter ----
            s_cur = pool.tile([P, P], bf16, tag="st_s")
            gsc = pool.tile([P, 17], f32, tag="st_gsc")
            with tc.For_i(0, nch1) as k:
                nc.vector.tensor_tensor(
                    out=s_cur[:],
                    in0=rl_t[:, bass.ds(k, 1)].to_broadcast([P, P]),
                    in1=iota_b[:], op=Alu.is_equal)
                gp = psum.tile([P, 17], f32, tag="gp")
                nc.tensor.matmul(out=gp[:], lhsT=s_cur[:],
                                 rhs=X_v[:, bass.ds(k, 1), :],
                                 start=True, stop=True)
                nc.vector.tensor_copy(out=gsc[:], in_=gp[:])
                nc.vector.tensor_copy(out=o1[:], in_=ob_t[:, bass.ds(k, 1)])
                nc.gpsimd.indirect_dma_start(
                    out=A_dram[:],
                    out_offset=bass.IndirectOffsetOnAxis(ap=o1[:], axis=0),
                    in_=gsc[:], in_offset=None, compute_op=Alu.add)

            # ---- load A, normalize into Y ----
            Y = pool.tile([P, NCH2 * 17], f32, tag="bigA")
            nc.sync.dma_start(
                out=Y[:].rearrange("p (q c) -> p q c", c=17),
                in_=A_dram[0:NBLK * SLOT2].rearrange("(q p) c -> p q c", p=P))
            Y_v = Y[:].rearrange("p (q c) -> p q c", c=17)
            cnt = pool.tile([P, NCH2], f32, tag="wA")
            nc.vector.tensor_copy(out=cnt[:], in_=Y_v[:, :, 16])
            nc.vector.tensor_scalar_max(cnt[:], cnt[:], 1.0)
            inv = pool.tile([P, NCH2], f32, tag="wB")
            nc.vector.reciprocal(out=inv[:], in_=cnt[:])
            nt = pool.tile([P, NCH2], f32, tag="wC")
            nc.vector.tensor_tensor(out=nt[:], in0=cnt[:], in1=inv[:],
                                    op=Alu.mult)
            nc.scalar.activation(out=nt[:], in_=nt[:], func=Act.Copy,
                                 bias=2.0, scale=-1.0)
            nc.vector.tensor_tensor(out=inv[:], in0=inv[:], in1=nt[:],
                                    op=Alu.mult)
            for c in range(16):
                nc.vector.tensor_tensor(out=Y_v[:, :, c], in0=Y_v[:, :, c],
                                        in1=inv[:], op=Alu.mult)
            nc.vector.tensor_copy(out=Y_v[:, :, 16], in_=vm_t[:])

            # ---- L3: stage-2 binning into M (PSUM start/stop, no scatter) ----
            M_sb = pool.tile([P, NBLK * 17], f32, tag="M")
            s2 = pool.tile([P, P], f32, tag="st_s2")
            with tc.For_i(0, NBLK) as b:
                mp = psum.tile([P, 17], f32, tag="mp")
                for t in range(2):
                    nc.vector.tensor_tensor(
                        out=s2[:],
                        in0=rl2_t[:, bass.ds(b * 2 + t, 1)].to_broadcast([P, P]),
                        in1=iota_f[:], op=Alu.is_equal)
                    nc.tensor.matmul(out=mp[:], lhsT=s2[:],
                                     rhs=Y_v[:, bass.ds(b * 2 + t, 1), :],
                                     start=(t == 0), stop=(t == 1))
                nc.vector.tensor_copy(out=M_sb[:, bass.ds(b * 17, 17)],
                                      in_=mp[:])

            # ---- M -> DRAM, ReduceScatter ----
            nc.sync.dma_start(
                out=M_dram[:].rearrange("(b p) c -> p b c", p=P),
                in_=M_sb[:].rearrange("p (b c) -> p b c", c=17))
            nc.gpsimd.collective_compute(
                "ReduceScatter", Alu.add, replica_groups=rg,
                ins=[M_dram[:]], outs=[Mred[:]])

            # ---- final: out = (Mred[:, :16]/deg) @ VT2 ----
            Mr = pool.tile([P, OQ * 17], f32, tag="Mr")
            nc.sync.dma_start(
                out=Mr[:].rearrange("p (q c) -> p q c", c=17),
                in_=Mred[:].rearrange("(q p) c -> p q c", p=P))
            Mr_v = Mr[:].rearrange("p (q c) -> p q c", c=17)
            deg = pool.tile([P, OQ], f32, tag="st_deg")
            nc.vector.tensor_copy(out=deg[:], in_=Mr_v[:, :, 16])
            nc.vector.tensor_scalar_max(deg[:], deg[:], 1.0)
            idg = pool.tile([P, OQ], f32, tag="st_idg")
            nc.vector.reciprocal(out=idg[:], in_=deg[:])
            nt2 = pool.tile([P, OQ], f32, tag="st_nt2")
            nc.vector.tensor_tensor(out=nt2[:], in0=deg[:], in1=idg[:],
                                    op=Alu.mult)
            nc.scalar.activation(out=nt2[:], in_=nt2[:], func=Act.Copy,
                                 bias=2.0, scale=-1.0)
            nc.vector.tensor_tensor(out=idg[:], in0=idg[:], in1=nt2[:],
                                    op=Alu.mult)

            m_cur = pool.tile([P, 17], f32, tag="st_mcur")
            tps = pool.tile([16, P], f32, tag="st_tps")
            ob_o = pool.tile([P, OUT_F], f32, tag="st_ob")
            out_v = out_o[:].rearrange("(q p) f -> p q f", p=P)
            with tc.For_i(0, OQ) as b:
                nc.vector.tensor_copy(out=m_cur[:], in_=Mr_v[:, bass.ds(b, 1), :])
                tp = psum.tile([16, P], f32, tag="tp")
                nc.tensor.transpose(out=tp[:], in_=m_cur[:, 0:16],
                                    identity=ident_t[:])
                nc.vector.tensor_copy(out=tps[:], in_=tp[:])
                op = psum.tile([P, OUT_F], f32, tag="op")
                nc.tensor.matmul(out=op[:], lhsT=tps[:], rhs=vt_t[:],
                                 start=True, stop=True)
                nc.vector.tensor_tensor(
                    out=ob_o[:], in0=op[:],
                    in1=idg[:, bass.ds(b, 1)].to_broadcast([P, OUT_F]),
                    op=Alu.mult)
                nc.sync.dma_start(out=out_v[:, bass.ds(b, 1), :], in_=ob_o[:])
    nc.compile()
    return nc


_CACHE = {}


def _pack_core(ci, g_src, lg_src, lg_dst, costheta, dnr, nch1):
    g0 = ci * GSLICE
    gs_loc = g_src[g0:g0 + GSLICE]
    order = np.argsort(gs_loc, kind="stable")          # rank -> local g idx
    sorted_nodes = gs_loc[order]
    blk_of_rank = sorted_nodes >> 7
    n_b = np.bincount(blk_of_rank, minlength=NBLK)
    assert n_b.max() <= SLOT2, f"core {ci}: node block overflow {n_b.max()}"
    cumstart = np.zeros(NBLK, np.int64)
    cumstart[1:] = np.cumsum(n_b)[:-1]
    j2 = np.arange(GSLICE) - cumstart[blk_of_rank]     # slot within block
    row2_of_rank = SLOT2 * blk_of_rank + j2
    rank_of_local = np.empty(GSLICE, np.int64)
    rank_of_local[order] = np.arange(GSLICE)

    # stage-2 slot maps [P, NCH2]
    nodeinb = sorted_nodes - (blk_of_rank << 7)
    rl2 = np.full((NBLK, SLOT2), 999.0, np.float32)
    rl2[blk_of_rank, j2] = nodeinb
    vm = np.zeros((NBLK, SLOT2), np.float32)
    vm[blk_of_rank, j2] = 1.0
    rowloc2 = np.ascontiguousarray(rl2.reshape(NCH2, P).T)
    vmask = np.ascontiguousarray(vm.reshape(NCH2, P).T)

    # edges owned by this core, sorted by dest row2
    sel = np.nonzero((lg_src >= g0) & (lg_src < g0 + GSLICE))[0]
    r_e = rank_of_local[lg_src[sel] - g0]
    eo = np.argsort(r_e, kind="stable")
    sel = sel[eo]
    row2_e = row2_of_rank[r_e[eo]]
    hb_e = row2_e >> 7                                  # half-block [0, 784)
    cnt_hb = np.bincount(hb_e, minlength=NCH2)
    nch_hb = (cnt_hb + 127) // 128
    nch = int(nch_hb.sum())
    assert nch <= nch1, f"core {ci}: {nch} chunks > {nch1}"
    hbstart = np.zeros(NCH2, np.int64)
    hbstart[1:] = np.cumsum(cnt_hb)[:-1]
    chstart = np.zeros(NCH2, np.int64)
    chstart[1:] = np.cumsum(nch_hb)[:-1]
    pos = np.arange(len(sel)) - hbstart[hb_e]
    chunk_e = chstart[hb_e] + (pos >> 7)
    lane_e = pos & 127

    def scat(vals, dtype, fill):
        arr = np.full((P, nch1), fill, dtype)
        arr[lane_e, chunk_e] = vals
        return arr

    e_lgs = lg_src[sel]
    e_lgd = lg_dst[sel]
    lgs_arr = scat((e_lgs // GSLICE) * GPC + e_lgs % GSLICE, np.int32, 0)
    lgd_arr = scat((e_lgd // GSLICE) * GPC + e_lgd % GSLICE, np.int32, 0)
    ct_arr = scat(costheta[sel], np.float32, 0.0)
    dnr_arr = scat(dnr[sel], np.float32, 0.0)
    rl_arr = scat((row2_e & 127).astype(np.float32), np.float32, 999.0)
    ob_row = np.zeros(nch1, np.int32)
    hb_of_chunk = np.repeat(np.arange(NCH2), nch_hb)
    ob_row[:nch] = (hb_of_chunk << 7).astype(np.int32)
    ob_arr = ob_row[None, :] + np.arange(P, dtype=np.int32)[:, None]

    return {
        "lgs": lgs_arr, "lgd": lgd_arr, "ct": ct_arr, "dnr": dnr_arr,
        "rloc": rl_arr, "obase": np.ascontiguousarray(ob_arr),
        "rloc2": rowloc2, "vmask": vmask,
    }


def kernel(atomic_number, g_src, g_dst, lg_src, lg_dst, costheta, dnr, a, b, c,
           d, value_table):
    atomic_number = np.asarray(atomic_number).astype(np.int64)
    g_src = np.asarray(g_src).astype(np.int64)
    g_dst = np.asarray(g_dst).astype(np.int64)
    lg_src = np.asarray(lg_src).astype(np.int64)
    lg_dst = np.asarray(lg_dst).astype(np.int64)
    costheta = np.asarray(costheta, dtype=np.float32)
    dnr = np.asarray(dnr, dtype=np.float32)
    a64 = np.asarray(a, dtype=np.float64)
    b64 = np.asarray(b, dtype=np.float64)
    c64 = np.asarray(c, dtype=np.float64)
    d64 = np.asarray(d, dtype=np.float64)
    value_table = np.asarray(value_table, dtype=np.float32)

    # spatial scalar constants: cos(a*theta + B) with theta = pi/2 - asin(x)
    # ~ q0 + q1*x + q2*x^2 for |x| <= EPS
    Ch = a64 * (math.pi / 2.0) + np.mod(b64, math.pi)
    cosC, sinC = np.cos(Ch), np.sin(Ch)
    sc = {
        "q0": [float(v) for v in (cosC + 1.0) / 2.0],
        "q1": [float(v) for v in (sinC / 2.0) * a64],
        "q2": [float(v) for v in (-cosC / 4.0) * a64 * a64],
        "c": [float(v) for v in c64],
        "d": [float(v) for v in d64],
    }

    # host packing (index-only work: shard, sort, slot/chunk layout)
    owner_cnt = np.bincount(lg_src // GSLICE, minlength=NCORES)
    # chunk count upper bound must be SPMD-uniform: compute exact per core
    packs = []
    nch_per_core = []
    for ci in range(NCORES):
        g0 = ci * GSLICE
        sel_mask = (lg_src >= g0) & (lg_src < g0 + GSLICE)
        # quick chunk count (same logic as _pack_core)
        gs_loc = g_src[g0:g0 + GSLICE]
        order = np.argsort(gs_loc, kind="stable")
        blk_of_rank = gs_loc[order] >> 7
        n_b = np.bincount(blk_of_rank, minlength=NBLK)
        cumstart = np.zeros(NBLK, np.int64)
        cumstart[1:] = np.cumsum(n_b)[:-1]
        row2_of_rank = SLOT2 * blk_of_rank + (np.arange(GSLICE) -
                                              cumstart[blk_of_rank])
        rank_of_local = np.empty(GSLICE, np.int64)
        rank_of_local[order] = np.arange(GSLICE)
        r_e = rank_of_local[lg_src[sel_mask] - g0]
        hb_e = row2_of_rank[r_e] >> 7
        cnt_hb = np.bincount(hb_e, minlength=NCH2)
        nch_per_core.append(int(((cnt_hb + 127) // 128).sum()))
    nch1 = max(nch_per_core)

    key = ("F", nch1) + tuple(sc["q0"] + sc["q1"] + sc["q2"] + sc["c"] + sc["d"])
    if key not in _CACHE:
        _CACHE[key] = build_fused(sc, nch1)
    nc = _CACHE[key]

    atab = np.zeros((GPC, 1), np.float32)
    atab[:N_NODES, 0] = atomic_number.astype(np.float32)
    VT2 = value_table.reshape(4, OUT_F, 4).transpose(0, 2, 1).reshape(16, OUT_F)
    VT2 = np.ascontiguousarray(VT2)

    in_maps = []
    for ci in range(NCORES):
        m = _pack_core(ci, g_src, lg_src, lg_dst, costheta, dnr, nch1)
        gsl = slice(ci * GSLICE, (ci + 1) * GSLICE)
        gpad = np.zeros(GPC, np.int32)
        gpad[:GSLICE] = g_src[gsl]
        m["gs"] = np.ascontiguousarray(gpad.reshape(GQ, P).T)
        gpad = np.zeros(GPC, np.int32)
        gpad[:GSLICE] = g_dst[gsl]
        m["gd"] = np.ascontiguousarray(gpad.reshape(GQ, P).T)
        m["atab"] = atab
        m["vt2"] = VT2
        in_maps.append(m)

    import time
    t0 = time.time()
    res = bass_utils.run_bass_kernel_spmd(nc, in_maps,
                                          core_ids=list(range(NCORES)))
    wall_ns = (time.time() - t0) * 1e9
    kernel.last_hw_ns = res.exec_time_ns if res.exec_time_ns else wall_ns

    out = np.concatenate([res.results[ci]["out"] for ci in range(NCORES)],
                         axis=0)
    return np.ascontiguousarray(out[:N_NODES]).astype(np.float32)


# revision 6
# speedup vs baseline: 4.4345x; 2.1902x over previous
"""Bass/Trainium2 kernel for nn_JP_Featurization (gnn_message_passing).

Single fused SPMD NEFF on 8 cores, one run_bass_kernel_spmd dispatch.
The axon tunnel moves ~23 MB/s, so the wire format is aggressively
quantized (u8/u16 indices and fixed-point ct/dnr, f16 output) and
everything derivable is computed on device.

Layout (per core ci):
  - g-edges range-sharded: slice [ci*50000, ci*50000+50000). Ranks =
    g-edges sorted by (g_src, g). Node block b (128 nodes) holds its
    ranks in SLOT2=256 slots: row2 = 256*blk + slot. pk table is built
    in row2 layout: each core gathers atomic[g_src], atomic[g_dst] per
    slot (u8 table), packs pk = ks + 4*kd, AllGather -> pk_full[8*100352].
  - lg edges sharded by owner of lg_src, sorted by dest row2, cut into
    128-edge chunks spanning <128 dest rows (greedy). Per chunk the host
    ships only a base row (b0, [1,nch1] i32, partition-broadcast on
    device via K=1 matmul); per edge: rloc u8 (row2 - b0), lgd split
    u8+u16 (pk row of lg_dst), ct/dnr u16 fixed-point. pk[lg_src] index
    = b0_global + rloc is derived on device.
  - Stage 1: For_i over chunks: one-hot(rloc) matmul bins 128 edges x 17
    payload channels onto 128 contiguous dest rows; indirect-DMA scatter
    (base+iota offsets, duplicate-free within a DMA) CCE-adds into
    A[100480,17] in DRAM.
  - Stage 2: ranks sorted by g_src => node block b's rows are chunks
    2b,2b+1; one-hot matmuls accumulate in PSUM (start/stop) into
    M[node,17]; no scatter. ReduceScatter sums M and shards it.
  - Final: per 128-node block, (M[:, :16]/deg) @ VT2 -> out f16 shard.
All chunk loops are hardware For_i loops (BIR stays ~800 instructions;
it is re-serialized on every jit trace, so size matters).
"""
import math

import numpy as np

import concourse.bass as bass
import concourse.bacc as bacc
import concourse.mybir as mybir
from concourse.tile import TileContext
from concourse.masks import make_identity
from concourse import bass_utils

P = 128
NCORES = 8
N_NODES = 50000
N_G = 400000
OUT_F = 64
EPS = 0.001

GSLICE = N_G // NCORES      # 50000 g-edges per core
NBLK = 392                  # node blocks of 128 (50176 >= 50000)
SLOT2 = 256                 # rank slots per node block
NSLOT = NBLK * SLOT2        # 100352 row2 slots per core
GQ2 = NSLOT // P            # 784 slot columns
GT = NSLOT * NCORES         # full pk table rows (802816)
NCH2 = NBLK * 2             # stage-2 chunks (784)
AROWS = NSLOT + P           # A accumulator rows (+128 scatter overrun pad)
ORPC = 6272                 # output rows per core (49*128); 8*6272 = 50176
OQ = ORPC // P              # 49
QS = 1.0 / 65536.0          # u16 fixed-point scale

f32 = mybir.dt.float32
f16 = mybir.dt.float16
bf16 = mybir.dt.bfloat16
u8 = mybir.dt.uint8
u16 = mybir.dt.uint16
i32 = mybir.dt.int32
Alu = mybir.AluOpType
Act = mybir.ActivationFunctionType


def build_fused(sc, nch1):
    nc = bacc.Bacc("TRN2", target_bir_lowering=False, debug=False,
                   num_devices=NCORES)
    atab = nc.dram_tensor("atab", [GQ2 * P // 2, 1], u8, kind="ExternalInput")
    gs = nc.dram_tensor("gs", [P, GQ2], u16, kind="ExternalInput")
    gd = nc.dram_tensor("gd", [P, GQ2], u16, kind="ExternalInput")
    lgdh = nc.dram_tensor("lgdh", [P, nch1], u8, kind="ExternalInput")
    lgdl = nc.dram_tensor("lgdl", [P, nch1], u16, kind="ExternalInput")
    ctq = nc.dram_tensor("ctq", [P, nch1], u16, kind="ExternalInput")
    dnq = nc.dram_tensor("dnq", [P, nch1], u16, kind="ExternalInput")
    rlq = nc.dram_tensor("rlq", [P, nch1], u8, kind="ExternalInput")
    b0g = nc.dram_tensor("b0g", [1, nch1], i32, kind="ExternalInput")
    b0l = nc.dram_tensor("b0l", [1, nch1], i32, kind="ExternalInput")
    rl2q = nc.dram_tensor("rl2q", [P, NCH2], u8, kind="ExternalInput")
    vt2 = nc.dram_tensor("vt2", [16, OUT_F], f32, kind="ExternalInput")
    out_o = nc.dram_tensor("out", [ORPC, OUT_F], f16, kind="ExternalOutput")

    rg = [list(range(NCORES))]

    with TileContext(nc) as tc:
        with (
            tc.tile_pool(name="sb", bufs=1) as pool,
            tc.tile_pool(name="psA", bufs=2, space="PSUM") as psA,
            tc.tile_pool(name="psB", bufs=1, space="PSUM") as psB,
            tc.tile_pool(name="dr", bufs=1, space="DRAM") as dram,
        ):
            # ---- device-generated constants ----
            iota_i = pool.tile([P, P], i32)
            nc.gpsimd.iota(iota_i[:], pattern=[[1, P]], base=0,
                           channel_multiplier=0)
            iota_b = pool.tile([P, P], bf16)
            nc.vector.tensor_copy(out=iota_b[:], in_=iota_i[:])
            iota_f = pool.tile([P, P], f32)
            nc.vector.tensor_copy(out=iota_f[:], in_=iota_i[:])
            pid_i = pool.tile([P, 1], i32)
            nc.gpsimd.iota(pid_i[:], pattern=[[0, 1]], base=0,
                           channel_multiplier=1)
            pid_f = pool.tile([P, 1], f32)
            nc.vector.tensor_copy(out=pid_f[:], in_=pid_i[:])
            ident_t = pool.tile([P, P], f32)
            make_identity(nc, ident_t[:])
            ones1 = pool.tile([1, P], f32)
            nc.vector.memset(ones1[:], 1.0)

            # ---- load inputs ----
            gs_t = pool.tile([P, GQ2], u16)
            gd_t = pool.tile([P, GQ2], u16)
            lgdh_t = pool.tile([P, nch1], u8)
            lgdl_t = pool.tile([P, nch1], u16)
            ctq_t = pool.tile([P, nch1], u16)
            dnq_t = pool.tile([P, nch1], u16)
            rlq_t = pool.tile([P, nch1], u8)
            b0g_t = pool.tile([1, nch1], i32)
            b0l_t = pool.tile([1, nch1], i32)
            rl2q_t = pool.tile([P, NCH2], u8)
            vt_t = pool.tile([16, OUT_F], f32)
            for t, s in ((gs_t, gs), (gd_t, gd), (lgdh_t, lgdh),
                         (lgdl_t, lgdl), (ctq_t, ctq), (dnq_t, dnq),
                         (rlq_t, rlq), (b0g_t, b0g), (b0l_t, b0l),
                         (rl2q_t, rl2q), (vt_t, vt2)):
                nc.sync.dma_start(out=t[:], in_=s[:])

            # ---- DRAM bounce tensors ----
            pk_slice = dram.tile([NSLOT, 1], f32, name="pk_slice")
            pk_full = dram.tile([GT, 1], f32, name="pk_full",
                                addr_space="Shared")
            A_dram = dram.tile([AROWS, 17], f32, name="A_dram")
            M_dram = dram.tile([NBLK * P, 17], f32, name="M_dram")
            Mred = dram.tile([ORPC, 17], f32, name="Mred")

            # ---- phase 0: atomic gathers in row2 slot order ----
            tmpw = pool.tile([P, GQ2], f32, tag="wT")
            gs32 = pool.tile([P, GQ2], i32)
            gd32 = pool.tile([P, GQ2], i32)
            nc.vector.tensor_copy(out=tmpw[:], in_=gs_t[:])
            nc.vector.tensor_copy(out=gs32[:], in_=tmpw[:])
            nc.vector.tensor_copy(out=tmpw[:], in_=gd_t[:])
            nc.vector.tensor_copy(out=gd32[:], in_=tmpw[:])
            ks8 = pool.tile([P, GQ2], u8)
            kd8 = pool.tile([P, GQ2], u8)
            o1 = pool.tile([P, 1], i32, tag="st_o1")
            o2 = pool.tile([P, 1], i32, tag="st_o2")
            g8a = pool.tile([P, 1], u8, tag="st_g8a")
            g8b = pool.tile([P, 1], u8, tag="st_g8b")
            with tc.For_i(0, GQ2, 2) as q:
                for u in range(2):
                    nc.vector.tensor_copy(out=o1[:],
                                          in_=gs32[:, bass.ds(q + u, 1)])
                    nc.gpsimd.indirect_dma_start(
                        out=g8a[:], out_offset=None, in_=atab[:],
                        in_offset=bass.IndirectOffsetOnAxis(ap=o1[:], axis=0))
                    nc.vector.tensor_copy(out=ks8[:, bass.ds(q + u, 1)],
                                          in_=g8a[:])
                    nc.vector.tensor_copy(out=o2[:],
                                          in_=gd32[:, bass.ds(q + u, 1)])
                    nc.gpsimd.indirect_dma_start(
                        out=g8b[:], out_offset=None, in_=atab[:],
                        in_offset=bass.IndirectOffsetOnAxis(ap=o2[:], axis=0))
                    nc.vector.tensor_copy(out=kd8[:, bass.ds(q + u, 1)],
                                          in_=g8b[:])
            ks_f = pool.tile([P, GQ2], f32)
            kd_f = pool.tile([P, GQ2], f32, tag="wT")
            nc.vector.tensor_copy(out=ks_f[:], in_=ks8[:])
            nc.vector.tensor_copy(out=kd_f[:], in_=kd8[:])
            nc.vector.tensor_scalar_mul(kd_f[:], kd_f[:], 4.0)
            nc.vector.tensor_tensor(out=ks_f[:], in0=ks_f[:], in1=kd_f[:],
                                    op=Alu.add)
            nc.sync.dma_start(
                out=pk_slice[:].rearrange("(q p) c -> p q c", p=P),
                in_=ks_f[:].rearrange("p (q c) -> p q c", c=1))
            nc.gpsimd.collective_compute(
                "AllGather", Alu.bypass, replica_groups=rg,
                ins=[pk_slice[:]], outs=[pk_full[:]])

            # ---- derive per-edge index arrays ----
            def bcast_row(src_i32, dst_f32):
                rowf = pool.tile([1, nch1], f32, tag="st_rowf")
                nc.vector.tensor_copy(out=rowf[:], in_=src_i32[:])
                for s in range(0, nch1, 512):
                    e = min(nch1, s + 512)
                    bp = psB.tile([P, 512], f32, tag="bp")
                    nc.tensor.matmul(out=bp[:, :e - s], lhsT=ones1[:],
                                     rhs=rowf[:, s:e], start=True, stop=True)
                    nc.vector.tensor_copy(out=dst_f32[:, s:e],
                                          in_=bp[:, :e - s])

            rl_f = pool.tile([P, nch1], f32)
            nc.vector.tensor_copy(out=rl_f[:], in_=rlq_t[:])
            rl_b = pool.tile([P, nch1], bf16)
            nc.vector.tensor_copy(out=rl_b[:], in_=rl_f[:])

            bcg = pool.tile([P, nch1], f32, tag="wA")
            bcast_row(b0g_t, bcg)
            nc.vector.tensor_tensor(out=bcg[:], in0=bcg[:], in1=rl_f[:],
                                    op=Alu.add)
            # pad lanes (rloc=255) could run past the pk table end: clamp
            nc.vector.tensor_scalar_min(bcg[:], bcg[:], float(GT - 1))
            lgs32 = pool.tile([P, nch1], i32)
            nc.vector.tensor_copy(out=lgs32[:], in_=bcg[:])

            bcl = pool.tile([P, nch1], f32, tag="wA")
            bcast_row(b0l_t, bcl)
            nc.vector.tensor_tensor(
                out=bcl[:], in0=bcl[:],
                in1=pid_f[:, 0:1].to_broadcast([P, nch1]), op=Alu.add)
            ob32 = pool.tile([P, nch1], i32)
            nc.vector.tensor_copy(out=ob32[:], in_=bcl[:])

            hi_f = pool.tile([P, nch1], f32, tag="wA")
            nc.vector.tensor_copy(out=hi_f[:], in_=lgdh_t[:])
            nc.vector.tensor_scalar_mul(hi_f[:], hi_f[:], 65536.0)
            lo_f = pool.tile([P, nch1], f32, tag="wB")
            nc.vector.tensor_copy(out=lo_f[:], in_=lgdl_t[:])
            nc.vector.tensor_tensor(out=hi_f[:], in0=hi_f[:], in1=lo_f[:],
                                    op=Alu.add)
            lgd32 = pool.tile([P, nch1], i32)
            nc.vector.tensor_copy(out=lgd32[:], in_=hi_f[:])

            # ---- L1: per-edge pk gathers ----
            pk1_t = pool.tile([P, nch1], f32)
            pk2_t = pool.tile([P, nch1], f32)
            g1 = pool.tile([P, 1], f32, tag="st_g1")
            g2 = pool.tile([P, 1], f32, tag="st_g2")
            with tc.For_i(0, nch1, 2) as k:
                for u in range(2):
                    nc.vector.tensor_copy(out=o1[:],
                                          in_=lgs32[:, bass.ds(k + u, 1)])
                    nc.gpsimd.indirect_dma_start(
                        out=g1[:], out_offset=None, in_=pk_full[:],
                        in_offset=bass.IndirectOffsetOnAxis(ap=o1[:], axis=0))
                    nc.vector.tensor_copy(out=pk1_t[:, bass.ds(k + u, 1)],
                                          in_=g1[:])
                    nc.vector.tensor_copy(out=o2[:],
                                          in_=lgd32[:, bass.ds(k + u, 1)])
                    nc.gpsimd.indirect_dma_start(
                        out=g2[:], out_offset=None, in_=pk_full[:],
                        in_offset=bass.IndirectOffsetOnAxis(ap=o2[:], axis=0))
                    nc.vector.tensor_copy(out=pk2_t[:, bass.ds(k + u, 1)],
                                          in_=g2[:])

            # ---- symmetry class ----
            kb_t = pool.tile([P, nch1], f32, tag="wA")
            kc_t = pool.tile([P, nch1], f32, tag="wB")
            tmp_t = pool.tile([P, nch1], f32, tag="wC")
            sym_t = pool.tile([P, nch1], f32, tag="wD")
            for dst, src in ((kb_t, pk1_t), (kc_t, pk2_t)):
                nc.vector.tensor_scalar(out=dst[:], in0=src[:], scalar1=4.0,
                                        scalar2=None, op0=Alu.is_ge)
                nc.vector.tensor_scalar(out=tmp_t[:], in0=src[:], scalar1=8.0,
                                        scalar2=None, op0=Alu.is_ge)
                nc.vector.tensor_tensor(out=dst[:], in0=dst[:], in1=tmp_t[:],
                                        op=Alu.add)
                nc.vector.tensor_scalar(out=tmp_t[:], in0=src[:], scalar1=12.0,
                                        scalar2=None, op0=Alu.is_ge)
                nc.vector.tensor_tensor(out=dst[:], in0=dst[:], in1=tmp_t[:],
                                        op=Alu.add)
            nc.vector.tensor_scalar_mul(tmp_t[:], kb_t[:], -4.0)
            nc.vector.tensor_tensor(out=pk1_t[:], in0=pk1_t[:], in1=tmp_t[:],
                                    op=Alu.add)  # ka
            nc.vector.tensor_tensor(out=sym_t[:], in0=pk1_t[:], in1=kc_t[:],
                                    op=Alu.is_equal)  # peripheral
            nc.vector.tensor_tensor(out=pk1_t[:], in0=kb_t[:], in1=pk1_t[:],
                                    op=Alu.is_equal)  # kb==ka
            nc.vector.tensor_tensor(out=kb_t[:], in0=kb_t[:], in1=kc_t[:],
                                    op=Alu.is_equal)  # kb==kc
            nc.vector.tensor_tensor(out=pk1_t[:], in0=pk1_t[:], in1=kb_t[:],
                                    op=Alu.mult)      # central
            nc.vector.tensor_scalar_mul(sym_t[:], sym_t[:], 2.0)
            nc.vector.tensor_tensor(out=sym_t[:], in0=sym_t[:], in1=pk1_t[:],
                                    op=Alu.add)

            # ---- spatial weights + payload X (bf16) ----
            x = pool.tile([P, nch1], f32, tag="wA")
            nc.vector.tensor_copy(out=x[:], in_=ctq_t[:])
            nc.vector.tensor_scalar_mul(x[:], x[:], QS)
            nc.vector.tensor_scalar_min(x[:], x[:], EPS)
            nc.vector.tensor_scalar_max(x[:], x[:], -EPS)
            dnr2 = pool.tile([P, nch1], f32, tag="wB")
            nc.vector.tensor_copy(out=dnr2[:], in_=dnq_t[:])
            nc.vector.tensor_scalar_mul(dnr2[:], dnr2[:], QS)
            nc.vector.tensor_tensor(out=dnr2[:], in0=dnr2[:], in1=dnr2[:],
                                    op=Alu.mult)
            x2_t = pool.tile([P, nch1], f32, tag="wC")
            nc.vector.tensor_tensor(out=x2_t[:], in0=x[:], in1=x[:],
                                    op=Alu.mult)
            X = pool.tile([P, nch1 * 17], bf16, tag="X")
            X_v = X[:].rearrange("p (q c) -> p q c", c=17)
            y_t = pool.tile([P, nch1], f32, tag="wE")
            t2_t = pool.tile([P, nch1], f32, tag="wF")
            m_t = pool.tile([P, nch1], f32, tag="wG")
            for h in range(4):
                nc.scalar.activation(out=y_t[:], in_=x[:], func=Act.Copy,
                                     bias=sc["q0"][h], scale=sc["q1"][h])
                nc.vector.tensor_scalar_mul(t2_t[:], x2_t[:], sc["q2"][h])
                nc.vector.tensor_tensor(out=y_t[:], in0=y_t[:], in1=t2_t[:],
                                        op=Alu.add)
                nc.scalar.activation(out=y_t[:], in_=y_t[:], func=Act.Ln,
                                     bias=0.0, scale=1.0)
                nc.vector.tensor_scalar_mul(y_t[:], y_t[:], sc["c"][h])
                nc.vector.tensor_scalar_mul(t2_t[:], dnr2[:], sc["d"][h])
                nc.vector.tensor_tensor(out=y_t[:], in0=y_t[:], in1=t2_t[:],
                                        op=Alu.subtract)
                nc.scalar.activation(out=y_t[:], in_=y_t[:], func=Act.Exp,
                                     bias=0.0, scale=1.0)
                for kk in range(4):
                    nc.vector.tensor_scalar(out=m_t[:], in0=sym_t[:],
                                            scalar1=float(kk), scalar2=None,
                                            op0=Alu.is_equal)
                    nc.vector.tensor_tensor(out=X_v[:, :, kk * 4 + h],
                                            in0=m_t[:], in1=y_t[:],
                                            op=Alu.mult)
            nc.vector.memset(X_v[:, :, 16], 1.0)

            # ---- zero A ----
            AQ = AROWS // P  # 786
            zt = pool.tile([P, AQ * 17], f32, tag="bigA")
            nc.vector.memset(zt[:], 0.0)
            nc.sync.dma_start(
                out=A_dram[:].rearrange("(q p) c -> p q c", p=P),
                in_=zt[:].rearrange("p (q c) -> p q c", c=17))

            # ---- L2: one-hot binning + contiguous CCE scatter ----
            s_cur = pool.tile([P, P], bf16, tag="st_s")
            gsc = pool.tile([P, 17], f32, tag="st_gsc")
            with tc.For_i(0, nch1, 2) as k:
                for u in range(2):
                    nc.vector.tensor_tensor(
                        out=s_cur[:],
                        in0=rl_b[:, bass.ds(k + u, 1)].to_broadcast([P, P]),
                        in1=iota_b[:], op=Alu.is_equal)
                    gp = psA.tile([P, 17], f32, tag="gp")
                    nc.tensor.matmul(out=gp[:], lhsT=s_cur[:],
                                     rhs=X_v[:, bass.ds(k + u, 1), :],
                                     start=True, stop=True)
                    nc.vector.tensor_copy(out=gsc[:], in_=gp[:])
                    nc.vector.tensor_copy(out=o1[:],
                                          in_=ob32[:, bass.ds(k + u, 1)])
                    nc.gpsimd.indirect_dma_start(
                        out=A_dram[:],
                        out_offset=bass.IndirectOffsetOnAxis(ap=o1[:], axis=0),
                        in_=gsc[:], in_offset=None, compute_op=Alu.add)

            # ---- load A, normalize into Y ----
            Y = pool.tile([P, NCH2 * 17], f32, tag="bigA")
            nc.sync.dma_start(
                out=Y[:].rearrange("p (q c) -> p q c", c=17),
                in_=A_dram[0:NSLOT].rearrange("(q p) c -> p q c", p=P))
            Y_v = Y[:].rearrange("p (q c) -> p q c", c=17)
            cnt = pool.tile([P, NCH2], f32, tag="wA")
            nc.vector.tensor_copy(out=cnt[:], in_=Y_v[:, :, 16])
            nc.vector.tensor_scalar_max(cnt[:], cnt[:], 1.0)
            inv = pool.tile([P, NCH2], f32, tag="wB")
            nc.vector.reciprocal(out=inv[:], in_=cnt[:])
            nt = pool.tile([P, NCH2], f32, tag="wC")
            nc.vector.tensor_tensor(out=nt[:], in0=cnt[:], in1=inv[:],
                                    op=Alu.mult)
            nc.scalar.activation(out=nt[:], in_=nt[:], func=Act.Copy,
                                 bias=2.0, scale=-1.0)
            nc.vector.tensor_tensor(out=inv[:], in0=inv[:], in1=nt[:],
                                    op=Alu.mult)
            for ch in range(16):
                nc.vector.tensor_tensor(out=Y_v[:, :, ch], in0=Y_v[:, :, ch],
                                        in1=inv[:], op=Alu.mult)
            rl2_f = pool.tile([P, NCH2], f32, tag="wD")
            nc.vector.tensor_copy(out=rl2_f[:], in_=rl2q_t[:])
            nc.vector.tensor_scalar(out=Y_v[:, :, 16], in0=rl2_f[:],
                                    scalar1=199.0, scalar2=None, op0=Alu.is_le)

            # ---- L3: stage-2 binning into M (PSUM start/stop) ----
            M_sb = pool.tile([P, NBLK * 17], f32, tag="M")
            s2 = pool.tile([P, P], f32, tag="st_s2")
            with tc.For_i(0, NBLK) as b:
                mp = psA.tile([P, 17], f32, tag="mp")
                for t in range(2):
                    nc.vector.tensor_tensor(
                        out=s2[:],
                        in0=rl2_f[:, bass.ds(b * 2 + t, 1)].to_broadcast([P, P]),
                        in1=iota_f[:], op=Alu.is_equal)
                    nc.tensor.matmul(out=mp[:], lhsT=s2[:],
                                     rhs=Y_v[:, bass.ds(b * 2 + t, 1), :],
                                     start=(t == 0), stop=(t == 1))
                nc.vector.tensor_copy(out=M_sb[:, bass.ds(b * 17, 17)],
                                      in_=mp[:])

            # ---- M -> DRAM, ReduceScatter ----
            nc.sync.dma_start(
                out=M_dram[:].rearrange("(b p) c -> p b c", p=P),
                in_=M_sb[:].rearrange("p (b c) -> p b c", c=17))
            nc.gpsimd.collective_compute(
                "ReduceScatter", Alu.add, replica_groups=rg,
                ins=[M_dram[:]], outs=[Mred[:]])

            # ---- final: out = (Mred[:, :16]/deg) @ VT2, f16 ----
            Mr = pool.tile([P, OQ * 17], f32, tag="Mr")
            nc.sync.dma_start(
                out=Mr[:].rearrange("p (q c) -> p q c", c=17),
                in_=Mred[:].rearrange("(q p) c -> p q c", p=P))
            Mr_v = Mr[:].rearrange("p (q c) -> p q c", c=17)
            deg = pool.tile([P, OQ], f32, tag="st_deg")
            nc.vector.tensor_copy(out=deg[:], in_=Mr_v[:, :, 16])
            nc.vector.tensor_scalar_max(deg[:], deg[:], 1.0)
            idg = pool.tile([P, OQ], f32, tag="st_idg")
            nc.vector.reciprocal(out=idg[:], in_=deg[:])
            nt2 = pool.tile([P, OQ], f32, tag="st_nt2")
            nc.vector.tensor_tensor(out=nt2[:], in0=deg[:], in1=idg[:],
                                    op=Alu.mult)
            nc.scalar.activation(out=nt2[:], in_=nt2[:], func=Act.Copy,
                                 bias=2.0, scale=-1.0)
            nc.vector.tensor_tensor(out=idg[:], in0=idg[:], in1=nt2[:],
                                    op=Alu.mult)

            m_cur = pool.tile([P, 17], f32, tag="st_mcur")
            tps = pool.tile([16, P], f32, tag="st_tps")
            ob_o = pool.tile([P, OUT_F], f16, tag="st_ob")
            out_v = out_o[:].rearrange("(q p) f -> p q f", p=P)
            with tc.For_i(0, OQ) as b:
                nc.vector.tensor_copy(out=m_cur[:], in_=Mr_v[:, bass.ds(b, 1), :])
                tp = psB.tile([16, P], f32, tag="tp")
                nc.tensor.transpose(out=tp[:], in_=m_cur[:, 0:16],
                                    identity=ident_t[:])
                nc.vector.tensor_copy(out=tps[:], in_=tp[:])
                op = psB.tile([P, OUT_F], f32, tag="op")
                nc.tensor.matmul(out=op[:], lhsT=tps[:], rhs=vt_t[:],
                                 start=True, stop=True)
                nc.vector.tensor_tensor(
                    out=ob_o[:], in0=op[:],
                    in1=idg[:, bass.ds(b, 1)].to_broadcast([P, OUT_F]),
                    op=Alu.mult)
                nc.sync.dma_start(out=out_v[:, bass.ds(b, 1), :], in_=ob_o[:])
    nc.compile()
    return nc


_CACHE = {}


def _greedy_chunks(row2_e):
    """Chunk starts: <=128 edges per chunk, dest span <128 rows."""
    starts = []
    i = 0
    n = len(row2_e)
    while i < n:
        starts.append(i)
        hi = min(i + P, n)
        lim = row2_e[i] + P
        j = i + int(np.searchsorted(row2_e[i:hi], lim, side="left"))
        i = j if j > i else i + 1
    return np.asarray(starts, np.int64)


def kernel(atomic_number, g_src, g_dst, lg_src, lg_dst, costheta, dnr, a, b, c,
           d, value_table):
    atomic_number = np.asarray(atomic_number).astype(np.int64)
    g_src = np.asarray(g_src).astype(np.int64)
    g_dst = np.asarray(g_dst).astype(np.int64)
    lg_src = np.asarray(lg_src).astype(np.int64)
    lg_dst = np.asarray(lg_dst).astype(np.int64)
    costheta = np.asarray(costheta, dtype=np.float32)
    dnr = np.asarray(dnr, dtype=np.float32)
    a64 = np.asarray(a, dtype=np.float64)
    b64 = np.asarray(b, dtype=np.float64)
    c64 = np.asarray(c, dtype=np.float64)
    d64 = np.asarray(d, dtype=np.float64)
    value_table = np.asarray(value_table, dtype=np.float32)

    Ch = a64 * (math.pi / 2.0) + np.mod(b64, math.pi)
    cosC, sinC = np.cos(Ch), np.sin(Ch)
    sc = {
        "q0": [float(v) for v in (cosC + 1.0) / 2.0],
        "q1": [float(v) for v in (sinC / 2.0) * a64],
        "q2": [float(v) for v in (-cosC / 4.0) * a64 * a64],
        "c": [float(v) for v in c64],
        "d": [float(v) for v in d64],
    }

    # ---- per-core rank/row2 structures ----
    row2_of_local = []
    loc_of_slot = []
    rl2_list = []
    for ci in range(NCORES):
        g0 = ci * GSLICE
        gs_loc = g_src[g0:g0 + GSLICE]
        order = np.argsort(gs_loc, kind="stable")
        sorted_nodes = gs_loc[order]
        blk = sorted_nodes >> 7
        n_b = np.bincount(blk, minlength=NBLK)
        assert n_b.max() <= SLOT2, f"core {ci}: node block overflow"
        cumstart = np.zeros(NBLK, np.int64)
        cumstart[1:] = np.cumsum(n_b)[:-1]
        j2 = np.arange(GSLICE) - cumstart[blk]
        row2_rank = SLOT2 * blk + j2
        rank_loc = np.empty(GSLICE, np.int64)
        rank_loc[order] = np.arange(GSLICE)
        row2_of_local.append(row2_rank[rank_loc])
        los = np.zeros(NSLOT, np.int64)
        los[row2_rank] = order
        loc_of_slot.append(los)
        rl2 = np.full((NBLK, SLOT2), 255, np.uint8)
        rl2[blk, j2] = (sorted_nodes - (blk << 7)).astype(np.uint8)
        rl2_list.append(np.ascontiguousarray(rl2.reshape(NCH2, P).T))
    glob_row2 = np.empty(N_G, np.int64)
    for ci in range(NCORES):
        glob_row2[ci * GSLICE:(ci + 1) * GSLICE] = (ci * NSLOT +
                                                    row2_of_local[ci])

    # ---- per-core edge packs ----
    owner = lg_src // GSLICE
    packs = []
    for ci in range(NCORES):
        g0 = ci * GSLICE
        sel = np.nonzero(owner == ci)[0]
        r2 = row2_of_local[ci][lg_src[sel] - g0]
        eo = np.argsort(r2, kind="stable")
        sel = sel[eo]
        row2_e = r2[eo]
        starts = _greedy_chunks(row2_e)
        nch = len(starts)
        cnts = np.diff(np.append(starts, len(sel)))
        chunk_e = np.repeat(np.arange(nch), cnts)
        lane_e = np.arange(len(sel)) - starts[chunk_e]
        b0 = row2_e[starts]
        packs.append(dict(sel=sel, row2_e=row2_e, b0=b0, chunk_e=chunk_e,
                          lane_e=lane_e, nch=nch))
    nch1 = max(p["nch"] for p in packs)
    nch1 = (nch1 + 1) & ~1  # even, for the 2x-unrolled For_i loops

    key = ("F2", nch1) + tuple(sc["q0"] + sc["q1"] + sc["q2"] + sc["c"] +
                               sc["d"])
    if key not in _CACHE:
        _CACHE[key] = build_fused(sc, nch1)
    nc = _CACHE[key]

    atab = np.zeros((NSLOT // 2, 1), np.uint8)
    atab[:N_NODES, 0] = atomic_number.astype(np.uint8)
    VT2 = value_table.reshape(4, OUT_F, 4).transpose(0, 2, 1).reshape(16, OUT_F)
    VT2 = np.ascontiguousarray(VT2)
    ctq_all = np.minimum(costheta * 65536.0, 65535.0).astype(np.uint16)
    dnq_all = np.minimum(dnr * 65536.0, 65535.0).astype(np.uint16)

    in_maps = []
    for ci in range(NCORES):
        pk = packs[ci]
        sel, chunk_e, lane_e = pk["sel"], pk["chunk_e"], pk["lane_e"]

        def scat(vals, dtype, fill):
            arr = np.full((P, nch1), fill, dtype)
            arr[lane_e, chunk_e] = vals
            return arr

        idx2 = glob_row2[lg_dst[sel]]
        b0row = np.zeros(nch1, np.int32)
        b0row[:pk["nch"]] = pk["b0"]
        m = {
            "atab": atab, "vt2": VT2, "rl2q": rl2_list[ci],
            "lgdh": scat((idx2 >> 16).astype(np.uint8), np.uint8, 0),
            "lgdl": scat((idx2 & 0xffff).astype(np.uint16), np.uint16, 0),
            "ctq": scat(ctq_all[sel], np.uint16, 0),
            "dnq": scat(dnq_all[sel], np.uint16, 0),
            "rlq": scat((pk["row2_e"] - pk["b0"][chunk_e]).astype(np.uint8),
                        np.uint8, 255),
            "b0l": b0row.reshape(1, nch1),
            "b0g": (b0row + np.int32(ci * NSLOT)).reshape(1, nch1),
        }
        g0 = ci * GSLICE
        los = loc_of_slot[ci]
        m["gs"] = np.ascontiguousarray(
            g_src[g0 + los].astype(np.uint16).reshape(GQ2, P).T)
        m["gd"] = np.ascontiguousarray(
            g_dst[g0 + los].astype(np.uint16).reshape(GQ2, P).T)
        in_maps.append(m)

    import time
    t0 = time.time()
    res = bass_utils.run_bass_kernel_spmd(nc, in_maps,
                                          core_ids=list(range(NCORES)))
    wall_ns = (time.time() - t0) * 1e9
    kernel.last_hw_ns = res.exec_time_ns if res.exec_time_ns else wall_ns

    out = np.concatenate([res.results[ci]["out"] for ci in range(NCORES)],
                         axis=0)
    return np.ascontiguousarray(out[:N_NODES]).astype(np.float32)
